# revision 1
# baseline (speedup 1.0000x reference)
"""MCR2 loss kernel for 8 Trainium2 NeuronCores.

Data-parallel over the sample axis: each core streams its 75000-row shard
of Z once, building per-128-sample-tile one-hot-masked copies of Z with a
single fused DVE scalar_tensor_tensor op (M[p, j*32+f] = (j == label_p) *
Z[p, f]) and accumulating Z_tile^T @ M_tile into PSUM, which yields all 10
per-class Grams Gj = Z^T diag(Pi_j) Z.  G = sum_j Gj exactly (one-hot).
The tiny [10,32,32] partials are summed on the host and the 32x32 logdets
are evaluated there in float64.
"""

import os
import sys

sys.path.insert(0, "/opt/trn_rl_repo")

import numpy as np

import concourse.bacc as bacc
import concourse.bass as bass
import concourse.mybir as mybir
import concourse.tile as tile
from concourse.bass_utils import run_bass_kernel_spmd

N, D, C = 600000, 32, 10
EPS = 0.5
NCORES = 8
PER = N // NCORES            # 75000 rows per core
PAD = ((PER + 127) // 128) * 128   # 75008
NTILES = PAD // 128          # 586 tiles of 128 samples
T_FULL = 32                  # tiles per chunk
FULL_CHUNKS = NTILES // T_FULL      # 9
T_TAIL = NTILES - FULL_CHUNKS * T_FULL  # 10
MW = C * D                   # 320: masked block width

_cache = {}


def _build_program():
    nc = bacc.Bacc(None)
    z_dram = nc.dram_tensor("Z", [PAD, D], mybir.dt.float32, kind="ExternalInput")
    lab_dram = nc.dram_tensor("labels", [PAD], mybir.dt.int32, kind="ExternalInput")
    out_dram = nc.dram_tensor("grams", [128, MW], mybir.dt.float32, kind="ExternalOutput")

    # class-index constant, value j repeated D times: [128, 320] bf16
    iota_np = np.tile(np.repeat(np.arange(C), D)[None, :], (128, 1)).astype(
        np.dtype("bfloat16") if hasattr(np, "bfloat16") else np.float32
    )
    # ml_dtypes bfloat16 via mybir numpy mapping
    import ml_dtypes

    iota_np = np.tile(np.arange(C)[None, :], (128, 1)).astype(ml_dtypes.bfloat16)
    iota_dram = nc.inline_tensor(iota_np, name="iota_c")

    bf16 = mybir.dt.bfloat16
    f32 = mybir.dt.float32

    with tile.TileContext(nc) as tc:
        with (
            tc.tile_pool(name="zraw", bufs=2) as zraw_pool,
            tc.tile_pool(name="zin", bufs=2) as zin_pool,
            tc.tile_pool(name="lab", bufs=2) as lab_pool,
            tc.tile_pool(name="labf", bufs=2) as labf_pool,
            tc.tile_pool(name="mask", bufs=2) as m_pool,
            tc.tile_pool(name="mask10", bufs=2) as mk_pool,
            tc.tile_pool(name="const", bufs=1) as const_pool,
            tc.tile_pool(name="outp", bufs=1) as out_pool,
            tc.tile_pool(name="psum", bufs=1, space="PSUM") as psum_pool,
        ):
            iota_sb = const_pool.tile([128, C], bf16)
            nc.sync.dma_start(iota_sb[:], iota_dram[:])
            # Tiny DVE read of the const so the DVE engine's vector clock
            # observes the const DMA once, instead of the wait landing on a
            # later STT (walrus: "Too many sync wait commands").
            touch = const_pool.tile([128, 2], bf16)
            nc.vector.tensor_copy(touch[:], iota_sb[:, 0:2])

            acc = psum_pool.tile([128, MW], f32)

            z_full = z_dram[0 : FULL_CHUNKS * 128 * T_FULL, :].rearrange(
                "(c p t) d -> c p (t d)", p=128, t=T_FULL
            )
            lab_full = lab_dram[0 : FULL_CHUNKS * 128 * T_FULL].rearrange(
                "(c p t) -> c p t", p=128, t=T_FULL
            )
            z_tail = z_dram[FULL_CHUNKS * 128 * T_FULL :, :].rearrange(
                "(p t) d -> p (t d)", p=128, t=T_TAIL
            )
            lab_tail = lab_dram[FULL_CHUNKS * 128 * T_FULL :].rearrange(
                "(p t) -> p t", p=128, t=T_TAIL
            )

            gtile = 0
            for c in range(FULL_CHUNKS + 1):
                tchunk = T_FULL if c < FULL_CHUNKS else T_TAIL
                z_raw = zraw_pool.tile([128, T_FULL * D], f32, tag="zr")
                z_sb = zin_pool.tile([128, T_FULL * D], bf16, tag="z")
                lab_sb = lab_pool.tile([128, T_FULL], mybir.dt.int32, tag="l")
                labf_sb = labf_pool.tile([128, T_FULL], bf16, tag="lf")
                if c < FULL_CHUNKS:
                    nc.sync.dma_start(z_raw[:, : tchunk * D], z_full[c])
                    nc.sync.dma_start(lab_sb[:, :tchunk], lab_full[c])
                else:
                    nc.sync.dma_start(z_raw[:, : tchunk * D], z_tail[:])
                    nc.sync.dma_start(lab_sb[:, :tchunk], lab_tail[:])
                nc.vector.tensor_copy(labf_sb[:, :tchunk], lab_sb[:, :tchunk])
                # fp32 -> bf16 cast on the otherwise-idle Scalar engine; also
                # the single sync point between the Z DMA and downstream readers.
                nc.scalar.mul(z_sb[:, : tchunk * D], z_raw[:, : tchunk * D], 1.0)

                # one-hot mask for the whole chunk: [128, t, j]
                mk_sb = mk_pool.tile([128, T_FULL * C], bf16, tag="mk")
                nc.vector.tensor_tensor(
                    out=mk_sb[:, : tchunk * C].rearrange("p (t j) -> p t j", j=C),
                    in0=labf_sb[:, :tchunk].unsqueeze(2).broadcast_to(
                        [128, tchunk, C]
                    ),
                    in1=iota_sb[:].unsqueeze(1).broadcast_to([128, tchunk, C]),
                    op=mybir.AluOpType.is_equal,
                )
                # masked copies for the whole chunk in one wide multiply:
                # M[p, t, j, f] = mask[p, t, j] * Z[p, t, f]
                m_sb = m_pool.tile([128, T_FULL * MW], bf16, tag="m")
                for eng, lo, hi in ((nc.vector, 0, tchunk),):
                    nt = hi - lo
                    eng.tensor_tensor(
                        out=m_sb[:, lo * MW : hi * MW].rearrange(
                            "p (t j f) -> p t j f", j=C, f=D
                        ),
                        in0=mk_sb[:, lo * C : hi * C]
                        .rearrange("p (t j) -> p t j", j=C)
                        .unsqueeze(3)
                        .broadcast_to([128, nt, C, D]),
                        in1=z_sb[:, lo * D : hi * D]
                        .rearrange("p (t f) -> p t f", f=D)
                        .unsqueeze(2)
                        .broadcast_to([128, nt, C, D]),
                        op=mybir.AluOpType.mult,
                    )
                for t in range(tchunk):
                    grp = gtile % 4
                    nc.tensor.matmul(
                        acc[grp * D : (grp + 1) * D, :],
                        z_sb[:, t * D : (t + 1) * D],
                        m_sb[:, t * MW : (t + 1) * MW],
                        start=(gtile < 4),
                        stop=(gtile >= NTILES - 4),
                        tile_position=(0, grp * D),
                    )
                    gtile += 1

            out_sb = out_pool.tile([128, MW], f32)
            nc.vector.tensor_copy(out_sb[:], acc[:])
            nc.sync.dma_start(out_dram[:], out_sb[:])

    nc.compile()
    return nc


def kernel(Z: np.ndarray, labels: np.ndarray) -> np.ndarray:
    Z = np.asarray(Z, dtype=np.float32)
    labels = np.asarray(labels, dtype=np.int32)

    if "nc" not in _cache:
        _cache["nc"] = _build_program()
    nc = _cache["nc"]

    in_maps = []
    for k in range(NCORES):
        zs = Z[k * PER : (k + 1) * PER]
        ls = labels[k * PER : (k + 1) * PER]
        zp = np.zeros([PAD, D], np.float32)
        zp[:PER] = zs
        lp = np.zeros([PAD], np.int32)
        lp[:PER] = ls
        in_maps.append({"Z": zp, "labels": lp})

    res = run_bass_kernel_spmd(nc, in_maps, core_ids=list(range(NCORES)))
    _cache["last_results"] = res

    gj = np.zeros([C, D, D], np.float64)
    for r in res.results:
        g = r["grams"].astype(np.float64).reshape(4, D, MW).sum(axis=0)
        for j in range(C):
            gj[j] += g[:, j * D : (j + 1) * D]

    g_all = gj.sum(axis=0)
    tr_pi = np.bincount(labels, minlength=C).astype(np.float64)

    nf, df = float(N), float(D)
    eye = np.eye(D)
    loss_r = 0.5 * np.linalg.slogdet(eye + (df / (nf * EPS)) * g_all)[1]
    loss_rc = 0.0
    for j in range(C):
        ld = np.linalg.slogdet(eye + (df / (tr_pi[j] * EPS)) * gj[j])[1]
        loss_rc += (tr_pi[j] / (2.0 * nf)) * ld
    loss_obj = loss_r - loss_rc
    return np.asarray([-loss_obj, loss_r, loss_rc], dtype=np.float32)



# revision 4
# speedup vs baseline: 5.7863x; 5.7863x over previous
"""MCR2 loss kernel for 8 Trainium2 NeuronCores.

Host-side counting sort by class label removes all masking work from the
device: each core receives its share of every class's rows, zero-padded
to 512-row (quad) alignment and pre-packed in bf16.  A quad is 4 sample
tiles of 128 rows laid side by side as a [128, 128] block Y; the device
computes Y^T @ Y, whose four diagonal [32,32] blocks are the four tiles'
Gram contributions (off-diagonal blocks are discarded).  One LDWEIGHTS
per 512 samples instead of per 128, and a 128-wide moving operand, keep
the PE at full utilization.  Per-class Grams accumulate across a class's
quads in PSUM; each finished accumulator is DMA'd straight to DRAM.
Host sums the diagonal blocks over cores in float64 and evaluates the
32x32 logdets there, exactly like the reference.
"""

import sys

sys.path.insert(0, "/opt/trn_rl_repo")

import ml_dtypes
import numpy as np

import concourse.bacc as bacc
import concourse.mybir as mybir
import concourse.tile as tile
from concourse.bass_utils import run_bass_kernel_spmd

N, D, C = 600000, 32, 10
EPS = 0.5
NCORES = 8
QC = 15                      # quads per (core, class): capacity 15*512 = 7680 rows
QROWS = 512                  # samples per quad (4 tiles of 128)
CW = QC * 128                # 1920 SBUF columns per class chunk
TOTW = C * CW                # 19200 columns of packed input per core

_cache = {}


def _build_program():
    nc = bacc.Bacc(None)
    bf16 = mybir.dt.bfloat16
    f32 = mybir.dt.float32

    z_dram = nc.dram_tensor("ZP", [128, TOTW], bf16, kind="ExternalInput")
    out_dram = nc.dram_tensor("grams", [128, C * 128], f32, kind="ExternalOutput")

    with tile.TileContext(nc) as tc:
        with (
            tc.tile_pool(name="zin", bufs=3) as zin_pool,
            tc.tile_pool(name="outp", bufs=2) as out_pool,
            tc.tile_pool(name="psum", bufs=4, space="PSUM") as psum_pool,
        ):
            z_sbs = []
            for j in range(min(3, C)):
                z_sb = zin_pool.tile([128, CW], bf16, tag="z")
                nc.sync.dma_start(z_sb[:], z_dram[:, j * CW : (j + 1) * CW])
                z_sbs.append(z_sb)

            for j in range(C):
                z_sb = z_sbs[j]
                acc = psum_pool.tile([128, 128], f32, tag="acc")
                for q in range(QC):
                    nc.tensor.matmul(
                        acc[:],
                        z_sb[:, q * 128 : (q + 1) * 128],
                        z_sb[:, q * 128 : (q + 1) * 128],
                        start=(q == 0),
                        stop=(q == QC - 1),
                    )
                if j + 3 < C:
                    z_nxt = zin_pool.tile([128, CW], bf16, tag="z")
                    nc.sync.dma_start(
                        z_nxt[:], z_dram[:, (j + 3) * CW : (j + 4) * CW]
                    )
                    z_sbs.append(z_nxt)
                o_sb = out_pool.tile([128, 128], f32, tag="o")
                nc.vector.tensor_copy(o_sb[:], acc[:])
                nc.sync.dma_start(out_dram[:, j * 128 : (j + 1) * 128], o_sb[:])

    nc.compile()
    return nc


def kernel(Z: np.ndarray, labels: np.ndarray) -> np.ndarray:
    Z = np.asarray(Z, dtype=np.float32)
    labels = np.asarray(labels, dtype=np.int32)
    n = Z.shape[0]

    if "nc" not in _cache:
        _cache["nc"] = _build_program()
    nc = _cache["nc"]

    counts = np.bincount(labels, minlength=C)
    assert counts.max() <= NCORES * QC * QROWS, "class capacity exceeded"
    order = np.argsort(labels, kind="stable")
    Zs = Z[order].astype(ml_dtypes.bfloat16)
    starts = np.concatenate([[0], np.cumsum(counts)])

    in_maps = []
    for k in range(NCORES):
        buf = np.zeros([C, QC * QROWS, D], ml_dtypes.bfloat16)
        for j in range(C):
            lo = starts[j] + k * counts[j] // NCORES
            hi = starts[j] + (k + 1) * counts[j] // NCORES
            buf[j, : hi - lo] = Zs[lo:hi]
        # pack: class j, quad q -> Y[p, 32*t+f] = rows[q*512 + t*128 + p, f]
        a = (
            buf.reshape(C, QC, 4, 128, D)
            .transpose(3, 0, 1, 2, 4)
            .reshape(128, TOTW)
        )
        in_maps.append({"ZP": np.ascontiguousarray(a)})

    res = run_bass_kernel_spmd(nc, in_maps, core_ids=list(range(NCORES)))
    _cache["last_results"] = res

    gj = np.zeros([C, D, D], np.float64)
    for r in res.results:
        g = r["grams"].astype(np.float64).reshape(128, C, 128)
        for j in range(C):
            for t in range(4):
                gj[j] += g[t * D : (t + 1) * D, j, t * D : (t + 1) * D]

    g_all = gj.sum(axis=0)
    tr_pi = counts.astype(np.float64)

    nf, df = float(n), float(D)
    eye = np.eye(D)
    loss_r = 0.5 * np.linalg.slogdet(eye + (df / (nf * EPS)) * g_all)[1]
    loss_rc = 0.0
    for j in range(C):
        ld = np.linalg.slogdet(eye + (df / (tr_pi[j] * EPS)) * gj[j])[1]
        loss_rc += (tr_pi[j] / (2.0 * nf)) * ld
    loss_obj = loss_r - loss_rc
    return np.asarray([-loss_obj, loss_r, loss_rc], dtype=np.float32)


# revision 5
# speedup vs baseline: 6.1156x; 1.0569x over previous
"""MCR2 loss kernel for 8 Trainium2 NeuronCores.

Host-side counting sort by class label removes all masking work from the
device: each core receives its share of every class's rows, zero-padded
to 512-row (quad) alignment and pre-packed in bf16.  A quad is 4 sample
tiles of 128 rows laid side by side as a [128, 128] block Y; the device
computes Y^T @ Y, whose four diagonal [32,32] blocks are the four tiles'
Gram contributions (off-diagonal blocks are discarded).  One LDWEIGHTS
per 512 samples instead of per 128, and a 128-wide moving operand, keep
the PE at full utilization.  Per-class Grams accumulate across a class's
quads in PSUM; each finished accumulator is DMA'd straight to DRAM.
Host sums the diagonal blocks over cores in float64 and evaluates the
32x32 logdets there, exactly like the reference.
"""

import sys

sys.path.insert(0, "/opt/trn_rl_repo")

import ml_dtypes
import numpy as np

import concourse.bacc as bacc
import concourse.mybir as mybir
import concourse.tile as tile
from concourse.bass_utils import run_bass_kernel_spmd

N, D, C = 600000, 32, 10
EPS = 0.5
NCORES = 8
QC = 15                      # quads per (core, class): capacity 15*512 = 7680 rows
QROWS = 512                  # samples per quad (4 tiles of 128)
CW = QC * 128                # 1920 SBUF columns per class chunk
TOTW = C * CW                # 19200 columns of packed input per core

_cache = {}


def _build_program():
    nc = bacc.Bacc(None)
    bf16 = mybir.dt.bfloat16
    f32 = mybir.dt.float32

    z_dram = nc.dram_tensor("ZP", [128, TOTW], bf16, kind="ExternalInput")
    out_dram = nc.dram_tensor("grams", [128, C * 128], f32, kind="ExternalOutput")

    # chunk schedule in quads: small first chunks so compute starts early,
    # then one chunk per class; all prefetched upfront (no slot reuse).
    chunks = [5, 5, 5] + [QC] * (C - 1)

    with tile.TileContext(nc) as tc:
        with (
            tc.tile_pool(name="zin", bufs=1) as zin_pool,
            tc.tile_pool(name="warm", bufs=1) as warm_pool,
            tc.tile_pool(name="outp", bufs=2) as out_pool,
            tc.tile_pool(name="psum", bufs=4, space="PSUM") as psum_pool,
        ):
            # PE p-state warmup: matmul a memset tile while DMAs stream.
            w_sb = warm_pool.tile([128, 128], bf16)
            nc.vector.memset(w_sb[:], 0)
            wacc = psum_pool.tile([128, 32], f32, tag="warm", bufs=1)
            for _ in range(16):
                nc.tensor.matmul(
                    wacc[:], w_sb[:], w_sb[:, 0:32], start=True, stop=True
                )

            # issue every input DMA upfront from the otherwise-idle GpSimd
            # sequencer; each chunk gets its own SBUF tile.
            z_tiles = []  # (tile, quad offset within tile) per global quad
            qoff = 0
            for ci, nq in enumerate(chunks):
                z_sb = zin_pool.tile(
                    [128, nq * 128], bf16, tag=f"z{ci}", bufs=1, name=f"zc{ci}"
                )
                nc.gpsimd.dma_start(
                    z_sb[:], z_dram[:, qoff * 128 : (qoff + nq) * 128]
                )
                for q in range(nq):
                    z_tiles.append((z_sb, q))
                qoff += nq

            for j in range(C):
                acc = psum_pool.tile([128, 128], f32, tag="acc")
                for q in range(QC):
                    z_sb, qo = z_tiles[j * QC + q]
                    nc.tensor.matmul(
                        acc[:],
                        z_sb[:, qo * 128 : (qo + 1) * 128],
                        z_sb[:, qo * 128 : (qo + 1) * 128],
                        start=(q == 0),
                        stop=(q == QC - 1),
                    )
                o_sb = out_pool.tile([128, 128], f32, tag="o")
                nc.vector.tensor_copy(o_sb[:], acc[:])
                nc.sync.dma_start(out_dram[:, j * 128 : (j + 1) * 128], o_sb[:])

    nc.compile()
    return nc


def kernel(Z: np.ndarray, labels: np.ndarray) -> np.ndarray:
    Z = np.asarray(Z, dtype=np.float32)
    labels = np.asarray(labels, dtype=np.int32)
    n = Z.shape[0]

    if "nc" not in _cache:
        _cache["nc"] = _build_program()
    nc = _cache["nc"]

    counts = np.bincount(labels, minlength=C)
    assert counts.max() <= NCORES * QC * QROWS, "class capacity exceeded"
    order = np.argsort(labels, kind="stable")
    Zs = Z[order].astype(ml_dtypes.bfloat16)
    starts = np.concatenate([[0], np.cumsum(counts)])

    in_maps = []
    for k in range(NCORES):
        buf = np.zeros([C, QC * QROWS, D], ml_dtypes.bfloat16)
        for j in range(C):
            lo = starts[j] + k * counts[j] // NCORES
            hi = starts[j] + (k + 1) * counts[j] // NCORES
            buf[j, : hi - lo] = Zs[lo:hi]
        # pack: class j, quad q -> Y[p, 32*t+f] = rows[q*512 + t*128 + p, f]
        a = (
            buf.reshape(C, QC, 4, 128, D)
            .transpose(3, 0, 1, 2, 4)
            .reshape(128, TOTW)
        )
        in_maps.append({"ZP": np.ascontiguousarray(a)})

    res = run_bass_kernel_spmd(nc, in_maps, core_ids=list(range(NCORES)))
    _cache["last_results"] = res

    gj = np.zeros([C, D, D], np.float64)
    for r in res.results:
        g = r["grams"].astype(np.float64).reshape(128, C, 128)
        for j in range(C):
            for t in range(4):
                gj[j] += g[t * D : (t + 1) * D, j, t * D : (t + 1) * D]

    g_all = gj.sum(axis=0)
    tr_pi = counts.astype(np.float64)

    nf, df = float(n), float(D)
    eye = np.eye(D)
    loss_r = 0.5 * np.linalg.slogdet(eye + (df / (nf * EPS)) * g_all)[1]
    loss_rc = 0.0
    for j in range(C):
        ld = np.linalg.slogdet(eye + (df / (tr_pi[j] * EPS)) * gj[j])[1]
        loss_rc += (tr_pi[j] / (2.0 * nf)) * ld
    loss_obj = loss_r - loss_rc
    return np.asarray([-loss_obj, loss_r, loss_rc], dtype=np.float32)
